# revision 1
# baseline (speedup 1.0000x reference)
"""Gaussian falloff vortex-velocity kernel for Trainium2 (Bass/Tile).

Math per batch element b (single vortex y,x,tau,sig per batch):
    d1 = py - y;  d2 = px - x;  q = d1^2 + d2^2
    s  = tau * exp(-q/sig^2) / sqrt(q)
    out[..., 0] = s * d2;  out[..., 1] = -s * d1

On-chip formulation (per core: 8 batches, each [512,512,2] -> [128, 4096]):
    De  = y - py                      (ACT Identity: scale=-1, bias=y)     = -d1
    Do  = px - x                      (DVE tensor_scalar_sub)              =  d2
    Qe  = Square(De * (1/sig))        (ACT Square with AP scale)           = d1^2/sig^2
    Qo  = Square(Do * (1/sig))
    q'  = Qe + Qo                     (DVE tensor_tensor add)              = q/sig^2
    L   = Ln(q')                      (ACT Ln)
    z   = 0.5*L + q'                  (DVE scalar_tensor_tensor)
    s'  = Exp(-z + ln(tau/sig))       (ACT Exp, imm scale=-1, AP bias)
        = tau/sig * exp(-q') / sqrt(q') = tau * exp(-q/sig^2) / sqrt(q)
    out_even = s' * Do;  out_odd = s' * De   (DVE tensor_tensor, strided writes)

All ACT functions (identity, square, ln, exp) live in the single
`natural_log_exp_and_others` table set -> one table load.
"""

import numpy as np

import concourse.bass as bass
import concourse.bacc as bacc
import concourse.mybir as mybir
from concourse.tile import TileContext
from concourse.bass_utils import run_bass_kernel_spmd
from concourse.hw_specs import get_activation_tables

N_CORES = 8
B_PER_CORE = 8          # 64 batches / 8 cores
P = 128                 # SBUF partitions
FD = 4096               # floats per partition for one batch ([512*512*2] / 128)
PTS = FD // 2           # points per partition
NCONST = 7              # y, x, g, -y*g, -x*g, 2/(sig*g)^2, ln(tau*g)
                        # g = 2^round(log2(1/sig)): power-of-two scaling makes
                        # y*g exact in fp32, so the Square's fused affine
                        # computes (py-y)*g with a single rounding (no
                        # catastrophic cancellation).

_PROGRAM = None


def _pin_act_table_set(arch: str):
    """Make all our activation functions resolve to the single
    `natural_log_exp_and_others` table set. The table-load inserter picks
    the FIRST set containing each function (Exp -> exp_and_others,
    Ln -> natural_log), which thrashes 2 table loads (~2.6us) per batch.
    get_activation_tables() is functools.cached and returns a mutable
    dict of sets; removing our functions from every other set (keeping
    indices intact) makes the combined set the unique first match."""
    AF = mybir.ActivationFunctionType
    try:
        tables = get_activation_tables(arch)
        keep = "natural_log_exp_and_others"
        needed = {AF.Identity, AF.Square, AF.Ln, AF.Exp, AF.Copy}
        if keep not in tables or not needed <= tables[keep]:
            return  # unexpected table layout: skip pinning (correct, slower)
        for name, fns in tables.items():
            if name != keep:
                fns -= needed
    except Exception:
        pass


def _stt_rev(eng, bass_obj, out, in0, scalar, in1, op0, op1):
    """scalar_tensor_tensor with reverse0: out = (scalar op0 in0) op1 in1.
    Same construction as BassEngine.scalar_tensor_tensor; reverse0 is in the
    ISA (and honored by HW) but not exposed by the bass wrapper."""
    return eng.add_instruction(
        mybir.InstTensorScalarPtr(
            name=bass_obj.get_next_instruction_name(),
            is_scalar_tensor_tensor=True,
            op0=op0,
            op1=op1,
            reverse0=True,
            ins=[eng.lower_ap(in0), eng.lower_ap_or_imm(scalar), eng.lower_ap(in1)],
            outs=[eng.lower_ap(out)],
        )
    )


def _build_program():
    f32 = mybir.dt.float32
    AF = mybir.ActivationFunctionType
    OP = mybir.AluOpType

    nc = bacc.Bacc(
        "TRN2",
        target_bir_lowering=False,
        debug=False,
        num_devices=N_CORES,
    )
    _pin_act_table_set(nc.m.arch)
    pts = nc.declare_dram_parameter("points", [B_PER_CORE * P, FD], f32, isOutput=False)
    cst = nc.declare_dram_parameter("consts", [P, NCONST * B_PER_CORE], f32, isOutput=False)
    out = nc.declare_dram_parameter("out", [B_PER_CORE * P, FD], f32, isOutput=True)

    with TileContext(nc) as tc:
        with (
            tc.tile_pool(name="cpool", bufs=1) as cpool,
            tc.tile_pool(name="tp", bufs=6) as tp,      # T tiles, 2MB each
            tc.tile_pool(name="qp", bufs=4) as qp,      # e tiles, 1MB each
            tc.tile_pool(name="qq", bufs=3) as qq,      # q tiles, 1MB each
            tc.tile_pool(name="op", bufs=2) as op_pool,  # O tiles, 2MB each
            tc.tile_pool(name="oph", bufs=2) as oph_pool,  # half-item O tiles, 1MB
        ):
            # Consts first on the sync ring: 3KB, lands ~1us after the ring
            # starts, ahead of the first 2MB T load on the same ring.
            c = cpool.tile([P, NCONST * B_PER_CORE], f32)
            nc.sync.dma_start(c[:], cst[:])

            # Warm-up activation with no dependencies: walrus inserts the ACT
            # table load (natural_log_exp_and_others) before the first
            # activation; doing it here keeps the load off the critical path
            # and away from wait-heavy instructions (HW wait-slot limit).
            w = cpool.tile([P, 1], f32)
            nc.vector.memset(w[:], 1.0)
            nc.scalar.activation(w[:], w[:], AF.Exp)

            def cap(b, j):
                return c[:, NCONST * b + j : NCONST * b + j + 1]

            # 3-stage software pipeline over work items (batch column-chunks):
            #   stage A (step i):   load T(i); Sq_e(i); Sq_o(i); q(i)=add
            #   stage B (step i+1): L(i)=Ln(q); z(i)=0.5L+q
            #   stage C (step i+2): s(i)=Exp(-z+lnts); out products; store
            # Emission order interleaves stages so neither ACT nor DVE ever
            # waits on the other within a step. First/last batches split in
            # halves to shorten pipeline fill (first compute needs only 1MB
            # of DMA) and drain (last store is 1MB and starts earlier).
            items = []
            for b in range(B_PER_CORE):
                if b in (0, B_PER_CORE - 1):
                    items.append((b, 0, FD // 2))
                    items.append((b, FD // 2, FD // 2))
                else:
                    items.append((b, 0, FD))
            Ts, Qs, qs = {}, {}, {}

            def stage_a(i):
                b, c0, w = items[i]
                rows = slice(b * P, (b + 1) * P)
                T = tp.tile([P, w], f32, tag="T")
                nc.sync.dma_start(T[:], pts[rows, c0 : c0 + w])
                Tv = T.rearrange("p (n c) -> p n c", c=2)
                e = qp.tile([P, w // 2], f32, tag="e")  # Qe, then L, then s
                q = qq.tile([P, w // 2], f32, tag="q")  # Qo, then q', then z
                Ts[i], Qs[i], qs[i] = Tv, e, q
                # Qe = ((py-y)/sig)^2 ; Qo = ((px-x)/sig)^2 (affine is fused FMA)
                nc.scalar.activation(e[:], Tv[:, :, 0], AF.Square, bias=cap(b, 3), scale=cap(b, 2))
                nc.scalar.activation(q[:], Tv[:, :, 1], AF.Square, bias=cap(b, 4), scale=cap(b, 2))
                nc.vector.tensor_tensor(q[:], q[:], e[:], OP.add)

            def stage_b(i):
                b = items[i][0]
                e, q = Qs[i], qs[i]
                nc.scalar.activation(e[:], q[:], AF.Ln)  # L = ln(u) over dead Qe
                # z2 = 2*alpha*u + L  (u in q; alpha = 1/(sig*g)^2)
                nc.vector.scalar_tensor_tensor(q[:], q[:], cap(b, 5), e[:], OP.mult, OP.add)

            def stage_c(i):
                b, c0, w = items[i]
                rows = slice(b * P, (b + 1) * P)
                Tv, e, q = Ts[i], Qs[i], qs[i]
                s = e[:]  # over dead L
                nc.scalar.activation(s, q[:], AF.Exp, bias=cap(b, 6), scale=-0.5)
                if w == FD:
                    O = op_pool.tile([P, w], f32, tag="O")
                else:
                    O = oph_pool.tile([P, w], f32, tag="Oh")
                Ov = O.rearrange("p (n c) -> p n c", c=2)
                # out_even = (px - x) * s ; out_odd = (y - py) * s
                nc.vector.scalar_tensor_tensor(Ov[:, :, 0], Tv[:, :, 1], cap(b, 1), s, OP.subtract, OP.mult)
                _stt_rev(nc.vector, nc, Ov[:, :, 1], Tv[:, :, 0], cap(b, 0), s, OP.subtract, OP.mult)
                nc.scalar.dma_start(out[rows, c0 : c0 + w], O[:])
                del Ts[i], Qs[i], qs[i]

            NI = len(items)
            for t in range(NI + 2):
                if t < NI:
                    stage_a(t)
                if 1 <= t <= NI:
                    stage_b(t - 1)
                if t >= 2:
                    stage_c(t - 2)

    nc.compile()
    return nc


def _get_program():
    global _PROGRAM
    if _PROGRAM is None:
        _PROGRAM = _build_program()
    return _PROGRAM


def _make_in_maps(vortex_feature, points):
    B, H, W, _ = points.shape
    vf = np.asarray(vortex_feature, dtype=np.float64).reshape(B, 6)
    y, x, tau, sig = vf[:, 0], vf[:, 1], vf[:, 2], vf[:, 3]
    sig_c = np.maximum(sig, 1e-35)  # sig==0 -> falloff 0; keep ln(tau*g) finite
    # Power-of-two scale g ~= 1/sig: y*g and x*g are exact fp32 products, so
    # the on-chip fused affine (p*g - y*g) has a single rounding.
    k = np.round(np.log2(1.0 / sig_c))
    g = np.exp2(k)
    two_alpha = 2.0 / (sig_c * g) ** 2  # in [0.5, 8); exp arg uses scale -0.5
    with np.errstate(divide="ignore"):
        lntg = np.log(tau) + k * np.log(2.0)  # ln(tau*g); tau==0 -> -inf (s'=0)
    consts = np.stack([y, x, g, -y * g, -x * g, two_alpha, lntg], axis=1).astype(np.float32)

    in_maps = []
    for i in range(N_CORES):
        sl = slice(i * B_PER_CORE, (i + 1) * B_PER_CORE)
        pshard = np.ascontiguousarray(points[sl]).reshape(B_PER_CORE * P, FD)
        cshard = np.ascontiguousarray(
            np.broadcast_to(consts[sl].reshape(1, NCONST * B_PER_CORE), (P, NCONST * B_PER_CORE))
        )
        in_maps.append({"points": pshard, "consts": cshard})
    return in_maps


def run(vortex_feature, points, trace=False, tmpdir=None):
    nc = _get_program()
    in_maps = _make_in_maps(vortex_feature, points)
    # The first execution of a freshly-loaded NEFF occasionally hits a
    # transient NRT_EXEC_UNIT_UNRECOVERABLE; a retry reliably succeeds.
    last_err = None
    for _ in range(3):
        try:
            res = run_bass_kernel_spmd(nc, in_maps, list(range(N_CORES)), trace=trace, tmpdir=tmpdir)
            break
        except Exception as err:  # noqa: BLE001
            last_err = err
    else:
        raise last_err
    B, H, W, _ = points.shape
    out = np.empty((B, H, W, 2), dtype=np.float32)
    for i in range(N_CORES):
        sl = slice(i * B_PER_CORE, (i + 1) * B_PER_CORE)
        out[sl] = res.results[i]["out"].reshape(B_PER_CORE, H, W, 2)
    return out, res


def kernel(vortex_feature: np.ndarray, points: np.ndarray) -> np.ndarray:
    out, _ = run(vortex_feature, points, trace=False)
    return out

